# revision 13
# baseline (speedup 1.0000x reference)
"""DiffPool GCN kernel for 8 Trainium2 NeuronCores.

Math (reference is PyG-style GCNConv x2 + softmax-assign pooling):
    Ahat = D^-1/2 (A + I) D^-1/2        (deg counts incoming edges + self loop)
    xa   = Ahat @ x                      # aggregate RAW features (linearity)
    z    = relu(xa @ w_embed + b_embed)
    s0   = relu(xa @ w_assign + b_assign)
    s    = softmax(s0 @ w_lin + b_lin, axis=-1)
    out  = s.T @ z                       # [512, 128]

Sharding: core d owns dst-nodes [d*6272, (d+1)*6272) of the 50176-padded
node set.  Each core aggregates its own nodes from the full x (resident in
its HBM), computes z/s locally and a partial pooled matmul; the host sums
the 8 partial [128hid, 512cl] outputs and transposes.

Device implementation per core:
  - per-edge gather of bf16 x rows from HBM via dma_gather (int16 indices,
    so src nodes are split into lo [0,32768) and hi [32768,50176) halves)
  - scatter-add to dst via one-hot matmuls on the tensor engine:
    xaT[ch, dst] += sum_e m_e[ch] * onehot(dstl_e), with DoubleRow bf16
    matmuls contracting 256 edges per instruction
  - two passes (lo then hi) accumulate xaT in SBUF
  - per 128-node block: z / s0T / logits matmuls, exp+accum on ACT,
    denominator folded into z, pooled matmul accumulated in PSUM
"""

import os
from contextlib import ExitStack

import numpy as np
import ml_dtypes

N_NODES = 50000
N_EDGES = 600000
IN_CH = 128
HID_CH = 128
ASSIGN_CH = 64
N_CLUSTERS = 512

NCORES = 8
P = 128
NPAD = 50176            # 8 * 6272
CORE_NODES = 6272       # 49 blocks of 128
NBLK = 49
HALF = 32768            # lo rows [0, HALF), hi rows [HALF, NPAD)
SLOT = 128              # edge slots per gather column
CHUNK_SLOTS = 1         # scatter-matmul chunk = 128 edges = 1 slot
GATHER_SLOTS = 64       # max slots per dma_gather instruction (8192 idx)

BF16 = ml_dtypes.bfloat16

_last_results = None    # test.py introspects exec time from here


# ---------------------------------------------------------------------------
# Host-side preprocessing: shard + order edges, build per-core device inputs
# ---------------------------------------------------------------------------

def _prep_inputs(x, edge_index, w_embed, b_embed, w_assign, b_assign,
                 w_lin, b_lin):
    src = np.asarray(edge_index[0], dtype=np.int64)
    dst = np.asarray(edge_index[1], dtype=np.int64)

    deg = np.bincount(dst, minlength=N_NODES).astype(np.float64) + 1.0
    dinv = (1.0 / np.sqrt(deg)).astype(np.float32)          # [N]

    # full edge list = graph edges + self loops
    loops = np.arange(N_NODES, dtype=np.int64)
    e_src = np.concatenate([src, loops])
    e_dst = np.concatenate([dst, loops])
    e_nrm = (dinv[e_src] * dinv[e_dst]).astype(np.float32)

    core_of = e_dst // CORE_NODES
    dloc = e_dst - core_of * CORE_NODES
    blk = dloc // P
    dstl = (dloc - blk * P).astype(np.int32)                # 0..127
    half = (e_src >= HALF).astype(np.int32)                 # 0=lo, 1=hi
    rel_src = (e_src - half * HALF).astype(np.int32)        # int16-safe

    # order: core, half major, block minor  (stable so layout deterministic)
    order = np.lexsort((dstl, blk, half, core_of))
    core_of = core_of[order]
    blk_o = blk[order]
    half_o = half[order]
    dstl_o = dstl[order]
    rel_o = rel_src[order]
    nrm_o = e_nrm[order]

    # per (core, half, block) counts -> padded run lengths (slots of 128)
    counts = np.zeros((NCORES, 2, NBLK), dtype=np.int64)
    np.add.at(counts, (core_of, half_o, blk_o), 1)
    pad_edges = CHUNK_SLOTS * SLOT                           # 256
    runs_e = np.maximum(
        ((counts + pad_edges - 1) // pad_edges) * pad_edges, pad_edges)
    runs_e = runs_e.max(axis=0)                              # same for all cores
    run_slots = (runs_e // SLOT).astype(np.int64)            # [2, NBLK], even
    total_slots = int(run_slots.sum())
    half_slots = [int(run_slots[0].sum()), int(run_slots[1].sum())]

    # slot-stream offset of each (half, block) run
    run_off = np.zeros((2, NBLK), dtype=np.int64)
    acc = 0
    for h in range(2):
        for b in range(NBLK):
            run_off[h, b] = acc
            acc += run_slots[h, b]

    n_edge_slots = total_slots * SLOT
    idx_a = np.zeros((NCORES, n_edge_slots), dtype=np.int16)
    nrm_a = np.zeros((NCORES, n_edge_slots), dtype=np.float32)
    dstl_a = np.zeros((NCORES, n_edge_slots), dtype=np.float32)

    # fill per-core streams
    start = np.searchsorted(core_of, np.arange(NCORES + 1))
    for c in range(NCORES):
        ch = half_o[start[c]:start[c + 1]]
        cb = blk_o[start[c]:start[c + 1]]
        pos_in_run = np.zeros((2, NBLK), dtype=np.int64)
        # edges of one core arrive sorted by (half, blk): compute positions
        # via grouped cumcount
        key = ch * NBLK + cb
        first = np.r_[True, key[1:] != key[:-1]]
        grp_start_idx = np.flatnonzero(first)
        cum = np.arange(key.size) - np.repeat(grp_start_idx,
                                              np.diff(np.r_[grp_start_idx,
                                                            key.size]))
        slot_pos = run_off[ch, cb] * SLOT + cum
        idx_a[c, slot_pos] = rel_o[start[c]:start[c + 1]].astype(np.int16)
        nrm_a[c, slot_pos] = nrm_o[start[c]:start[c + 1]]
        dstl_a[c, slot_pos] = dstl_o[start[c]:start[c + 1]].astype(np.float32)

    # device layouts ---------------------------------------------------
    # edge slot s, lane e (0..127): tile[p, s] with p = e  (gather writes
    # idx i of an instruction to partition i%128, column i//128; our slots
    # are 128-aligned so lane==partition)
    def to_lane_tiles(a, dt):
        t = a.reshape(NCORES, total_slots, SLOT).transpose(0, 2, 1)
        return np.ascontiguousarray(t.astype(dt))

    nrm_t = to_lane_tiles(nrm_a, BF16)                       # [8,128,S]
    dstl_t = to_lane_tiles(dstl_a, BF16)                     # [8,128,S]

    # dma_gather index tile: [128, S*8] int16; index i of a gather that
    # starts at slot s0 lives at row (i%16), col (s0*8 + i//16); rows are
    # replicated 8x across the 128 partitions.
    idx16 = idx_a.reshape(NCORES, total_slots * SLOT // 16, 16)
    idx16 = idx16.transpose(0, 2, 1)                         # [8,16,S*8]
    idx_t = np.ascontiguousarray(np.tile(idx16, (1, 8, 1)))  # [8,128,S*8]

    xp = np.zeros((NPAD, IN_CH), dtype=BF16)
    xp[:N_NODES] = np.asarray(x, dtype=np.float32).astype(BF16)

    wcat = np.concatenate([np.asarray(w_embed, np.float32),
                           np.asarray(w_assign, np.float32)],
                          axis=1).astype(BF16)               # [128,192]
    wlin_aug = np.concatenate([np.asarray(w_lin, np.float32),
                               np.asarray(b_lin, np.float32)[None, :]],
                              axis=0).astype(BF16)           # [65,512]
    bz_row = np.asarray(b_embed, np.float32).astype(BF16)[None, :]  # [1,128]
    ones_row = np.ones((1, P), dtype=BF16)
    ba_col = np.asarray(b_assign, np.float32)[:, None].copy()       # [64,1]
    iota = np.tile(np.arange(P, dtype=np.float32).astype(BF16)[None, :],
                   (P, 1)).copy()                            # [128,128]

    masks = np.zeros((NCORES, P, NBLK), dtype=BF16)
    for c in range(NCORES):
        nid = (c * CORE_NODES + np.arange(NBLK)[None, :] * P
               + np.arange(P)[:, None])
        masks[c] = (nid < N_NODES).astype(BF16)

    in_maps = []
    for c in range(NCORES):
        in_maps.append({
            "xp": xp,
            "idx": idx_t[c],
            "nrm": nrm_t[c],
            "dstl": dstl_t[c],
            "wcat": wcat,
            "wlin": wlin_aug,
            "bz": bz_row,
            "ones": ones_row,
            "ba": ba_col,
            "iota": iota,
            "mask": masks[c],
        })

    cfg = {
        "total_slots": total_slots,
        "half_slots": half_slots,
        "run_slots": run_slots,          # [2, NBLK]
        "run_off": run_off,              # [2, NBLK]
    }
    return in_maps, cfg


# ---------------------------------------------------------------------------
# Device program
# ---------------------------------------------------------------------------

def _gather_pieces(cfg):
    """Split each half's slot stream into dma_gather instructions.

    Returns list of (half, abs_slot_start, nslots); cuts are at even slot
    offsets so DoubleRow chunks never straddle a piece.
    """
    pieces = []
    base = 0
    for h in range(2):
        n = cfg["half_slots"][h]
        s = 0
        while s < n:
            take = min(GATHER_SLOTS, n - s)
            pieces.append((h, base + s, take))
            s += take
        base += n
    return pieces


def build_program(cfg, stage=99):
    """stage: 1=loads+gather+DVE prep, 2=+scatter matmuls, 3=+head, 99=all."""
    import concourse.bass as bass
    import concourse.mybir as mybir
    import concourse.bacc as bacc
    import concourse.tile as tile

    f32 = mybir.dt.float32
    bf16 = mybir.dt.bfloat16
    i16 = mybir.dt.int16

    S_total = cfg["total_slots"]
    run_slots = cfg["run_slots"]
    run_off = cfg["run_off"]
    pieces = _gather_pieces(cfg)

    nc = bacc.Bacc("TRN2", target_bir_lowering=False, debug=False,
                   enable_asserts=False)

    d_xp = nc.dram_tensor("xp", [NPAD, IN_CH], bf16, kind="ExternalInput").ap()
    d_idx = nc.dram_tensor("idx", [P, S_total * 8], i16,
                           kind="ExternalInput").ap()
    d_nrm = nc.dram_tensor("nrm", [P, S_total], bf16,
                           kind="ExternalInput").ap()
    d_dstl = nc.dram_tensor("dstl", [P, S_total], bf16,
                            kind="ExternalInput").ap()
    d_wcat = nc.dram_tensor("wcat", [P, 192], bf16, kind="ExternalInput").ap()
    d_wlin = nc.dram_tensor("wlin", [65, 512], bf16, kind="ExternalInput").ap()
    d_bz = nc.dram_tensor("bz", [1, P], bf16, kind="ExternalInput").ap()
    d_ones = nc.dram_tensor("ones", [1, P], bf16, kind="ExternalInput").ap()
    d_ba = nc.dram_tensor("ba", [ASSIGN_CH, 1], f32, kind="ExternalInput").ap()
    d_iota = nc.dram_tensor("iota", [P, P], bf16, kind="ExternalInput").ap()
    d_mask = nc.dram_tensor("mask", [P, NBLK], bf16, kind="ExternalInput").ap()
    d_out = nc.dram_tensor("xpoolT", [P, N_CLUSTERS], f32,
                           kind="ExternalOutput").ap()

    # block id for each chunk position in the slot stream
    slot_block = np.zeros(S_total, dtype=np.int64)
    slot_half = np.zeros(S_total, dtype=np.int64)
    for h in range(2):
        for b in range(NBLK):
            o = int(run_off[h, b])
            n = int(run_slots[h, b])
            slot_block[o:o + n] = b
            slot_half[o:o + n] = h

    with tile.TileContext(nc) as tc, ExitStack() as ctx:
        cpool = ctx.enter_context(tc.tile_pool(name="const", bufs=1))

        t_idx = cpool.tile([P, S_total * 8], i16)
        t_nrm = cpool.tile([P, S_total], bf16)
        t_dstl = cpool.tile([P, S_total], bf16)
        t_wcat = cpool.tile([P, 192], bf16)
        t_wlin = cpool.tile([65, 512], bf16)
        t_bz = cpool.tile([1, P], bf16)
        t_ones = cpool.tile([1, P], bf16)
        t_ba = cpool.tile([ASSIGN_CH, 1], f32)
        t_iota = cpool.tile([P, P], bf16)
        t_mask = cpool.tile([P, NBLK], bf16)
        t_xaT = cpool.tile([P, NBLK * P], bf16)     # aggregated features^T

        for t, d in ((t_idx, d_idx), (t_nrm, d_nrm), (t_dstl, d_dstl),
                     (t_wcat, d_wcat), (t_wlin, d_wlin), (t_bz, d_bz),
                     (t_ones, d_ones), (t_ba, d_ba), (t_iota, d_iota),
                     (t_mask, d_mask)):
            nc.sync.dma_start(out=t[:], in_=d)

        # ---------------- phase 1: aggregate xaT ----------------
        gpool = ctx.enter_context(tc.tile_pool(name="gather", bufs=2))
        spool = ctx.enter_context(tc.tile_pool(name="onehot", bufs=2))
        app = ctx.enter_context(
            tc.tile_pool(name="xaT_psum", bufs=1, space="PSUM"))

        x_lo = d_xp[0:HALF, :]
        x_hi = d_xp[HALF:NPAD, :]

        sub = os.environ.get("KERNEL_SUBSTAGE", "")
        for pi, (h, s0, ns) in enumerate(pieces):
            gbuf = gpool.tile([P, GATHER_SLOTS, IN_CH], bf16, tag="g")
            sbuf = spool.tile([P, GATHER_SLOTS, P], bf16, tag="s")
            nidx = ns * SLOT
            if sub == "loads":
                break
            if not (sub == "gather1" and pi > 0):
                nc.gpsimd.dma_gather(
                    out_ap=gbuf[:, 0:ns, :],
                    in_ap=(x_lo if h == 0 else x_hi),
                    idxs_ap=t_idx[:, s0 * 8:(s0 + ns) * 8],
                    num_idxs=nidx,
                    num_idxs_reg=nidx,
                    elem_size=IN_CH,
                    single_packet=False,
                )
            if sub in ("gather1", "gatherall"):
                continue
            # messages m_e = nrm_e * x[src_e]
            if sub != "noeq":
                nc.vector.tensor_tensor(
                    out=gbuf[:, 0:ns, :],
                    in0=gbuf[:, 0:ns, :],
                    in1=t_nrm[:, s0:s0 + ns].to_broadcast([P, ns, IN_CH]),
                    op=mybir.AluOpType.mult,
                )
            if sub == "nonorm":
                continue
            # one-hot S[e, :] = (iota == dstl_e)
            nc.vector.tensor_tensor(
                out=sbuf[:, 0:ns, :],
                in0=t_iota[:][:, None, :].to_broadcast([P, ns, P]),
                in1=t_dstl[:, s0:s0 + ns].to_broadcast([P, ns, P]),
                op=mybir.AluOpType.is_equal,
            )
            if stage < 2:
                continue
            # chunk matmuls: xaT_psum[ch, node] += m^T(x)S, 128 edges each
            for j in range(0, ns, CHUNK_SLOTS):
                s_abs = s0 + j
                b = int(slot_block[s_abs])
                first = s_abs == int(run_off[h, b])
                last = (s_abs + CHUNK_SLOTS
                        == int(run_off[h, b] + run_slots[h, b]))
                if first:
                    t_acc = app.tile([P, P], f32, tag="xaT")
                nc.tensor.matmul(
                    out=t_acc[:],
                    lhsT=gbuf[:, j, :],
                    rhs=sbuf[:, j, :],
                    start=first,
                    stop=last,
                )
                if last:
                    dst = t_xaT[:, b * P:(b + 1) * P]
                    if h == 0:
                        nc.scalar.copy(out=dst, in_=t_acc[:])
                    else:
                        nc.vector.tensor_add(out=dst, in0=t_acc[:],
                                             in1=dst)

        # ---------------- phase 2: per-block head ----------------
        if stage < 3:
            t_dummy = cpool.tile([P, N_CLUSTERS], f32)
            nc.vector.memset(t_dummy[:], 0.0)
            if stage >= 2:
                nc.vector.tensor_copy(out=t_dummy[:, 0:P],
                                      in_=t_xaT[:, 0:P])
            nc.sync.dma_start(out=d_out, in_=t_dummy[:])
            nblk_run = 0
        else:
            nblk_run = NBLK
        zpp = ctx.enter_context(
            tc.tile_pool(name="z_psum", bufs=2, space="PSUM"))
        spp = ctx.enter_context(
            tc.tile_pool(name="s0_psum", bufs=2, space="PSUM"))
        lpp = ctx.enter_context(
            tc.tile_pool(name="logit_psum", bufs=2, space="PSUM"))
        ppp = ctx.enter_context(
            tc.tile_pool(name="pool_psum", bufs=1, space="PSUM"))
        hpool = ctx.enter_context(tc.tile_pool(name="head", bufs=2))

        t_pool = ppp.tile([P, N_CLUSTERS], f32, tag="xpool")

        for b in range(nblk_run):
            xaT_b = t_xaT[:, b * P:(b + 1) * P]

            # z = xa @ Wz + bz   -> [node, hid] in PSUM
            t_z = zpp.tile([P, P], f32, tag="z")
            nc.tensor.matmul(out=t_z[:], lhsT=xaT_b,
                             rhs=t_wcat[:, 0:HID_CH], start=True, stop=False)
            nc.tensor.matmul(out=t_z[:], lhsT=t_ones[:], rhs=t_bz[:],
                             start=False, stop=True)

            # s0T = Wa^T @ xaT -> [64, node] in PSUM
            t_s0 = spp.tile([ASSIGN_CH, P], f32, tag="s0")
            nc.tensor.matmul(out=t_s0[:], lhsT=t_wcat[:, HID_CH:192],
                             rhs=xaT_b, start=True, stop=True)

            # s0aug = [relu(s0T + ba); ones]  -> [65, node] bf16
            t_sa = hpool.tile([65, P], bf16, tag="s0aug")
            nc.vector.tensor_scalar(
                out=t_sa[0:ASSIGN_CH, :], in0=t_s0[:],
                scalar1=t_ba[:], scalar2=0.0,
                op0=mybir.AluOpType.add, op1=mybir.AluOpType.max)
            nc.vector.memset(t_sa[ASSIGN_CH:65, :], 1.0)

            # logits = s0aug^T @ wlin_aug -> [node, 512] PSUM
            t_lg = lpp.tile([P, N_CLUSTERS], f32, tag="logits")
            nc.tensor.matmul(out=t_lg[:], lhsT=t_sa[:], rhs=t_wlin[:],
                             start=True, stop=True)

            # e = exp(logits), den = rowsum(e)
            t_e = hpool.tile([P, N_CLUSTERS], bf16, tag="e")
            t_den = hpool.tile([P, 1], f32, tag="den")
            nc.scalar.activation(out=t_e[:], in_=t_lg[:],
                                 func=mybir.ActivationFunctionType.Exp,
                                 accum_out=t_den[:])

            # zdiv = relu(z) * (mask / den)
            t_rd = hpool.tile([P, 1], f32, tag="rden")
            nc.vector.reciprocal(out=t_rd[:], in_=t_den[:])
            t_rdm = hpool.tile([P, 1], f32, tag="rdenm")
            nc.vector.tensor_tensor(out=t_rdm[:], in0=t_rd[:],
                                    in1=t_mask[:, b:b + 1],
                                    op=mybir.AluOpType.mult)
            t_zd = hpool.tile([P, P], bf16, tag="zdiv")
            nc.vector.tensor_scalar(
                out=t_zd[:], in0=t_z[:],
                scalar1=0.0, scalar2=t_rdm[:],
                op0=mybir.AluOpType.max, op1=mybir.AluOpType.mult)

            # x_poolT += zdiv^T(x)e  -> [hid, 512]
            nc.tensor.matmul(out=t_pool[:], lhsT=t_zd[:], rhs=t_e[:],
                             start=(b == 0), stop=(b == NBLK - 1))

        if nblk_run:
            t_out = cpool.tile([P, N_CLUSTERS], f32)
            nc.scalar.copy(out=t_out[:], in_=t_pool[:])
            nc.sync.dma_start(out=d_out, in_=t_out[:])

    nc.compile()
    return nc


# ---------------------------------------------------------------------------
# Entry point
# ---------------------------------------------------------------------------

def kernel(x, edge_index, w_embed, b_embed, w_assign, b_assign,
           w_lin, b_lin):
    global _last_results
    from concourse import bass_utils

    in_maps, cfg = _prep_inputs(x, edge_index, w_embed, b_embed,
                                w_assign, b_assign, w_lin, b_lin)
    nc = build_program(cfg)

    trace = bool(int(os.environ.get("KERNEL_TRACE", "0")))
    res = bass_utils.run_bass_kernel_spmd(
        nc, in_maps, core_ids=list(range(NCORES)), trace=trace)
    _last_results = res

    acc = np.zeros((P, N_CLUSTERS), dtype=np.float64)
    for om in res.results:
        acc += om["xpoolT"].astype(np.float64)
    return np.ascontiguousarray(acc.T.astype(np.float32))


# revision 18
# speedup vs baseline: 6.8659x; 6.8659x over previous
"""DiffPool GCN kernel for 8 Trainium2 NeuronCores.

Math (reference is PyG-style GCNConv x2 + softmax-assign pooling):
    Ahat = D^-1/2 (A + I) D^-1/2        (deg counts incoming edges + self loop)
    xa   = Ahat @ x                      # aggregate RAW features (linearity)
    z    = relu(xa @ w_embed + b_embed)
    s0   = relu(xa @ w_assign + b_assign)
    s    = softmax(s0 @ w_lin + b_lin, axis=-1)
    out  = s.T @ z                       # [512, 128]

Sharding: core d owns dst-nodes [d*6272, (d+1)*6272) of the 50176-padded
node set.  Each core aggregates its own nodes from the full x (resident in
its HBM), computes z/s locally and a partial pooled matmul; the host sums
the 8 partial [128hid, 512cl] outputs and transposes.

Device implementation per core:
  - x is pre-scaled on the host: x' = dinv[n] * x[n]; the remaining dst
    factor dinv[dst] is applied to the aggregated xaT as a per-block
    column scale
  - per-edge gather of bf16 x' rows from HBM via dma_gather (int16
    indices; src nodes split into lo [0,32767) and hi [32767,50176)
    halves with a reserved zero row so pad slots contribute nothing)
  - scatter-add to dst via matmuls on the tensor engine.  Edges are
    rank-sorted per dst node: slot r holds the r-th edge of node p at
    partition p, so the scatter matrix is the constant identity.  Only
    overflow edges beyond rank R take the one-hot path
    (S = is_equal(iota, dstl) built on DVE).
  - two passes (lo then hi) accumulate xaT in SBUF
  - per 128-node block: z / s0T / logits matmuls, exp+accum on ACT,
    denominator folded into z, pooled matmul accumulated in PSUM
"""

import os
from contextlib import ExitStack

import numpy as np
import ml_dtypes

N_NODES = 50000
N_EDGES = 600000
IN_CH = 128
HID_CH = 128
ASSIGN_CH = 64
N_CLUSTERS = 512

NCORES = 8
P = 128
NPAD = 50176            # 8 * 6272
CORE_NODES = 6272       # 49 blocks of 128
NBLK = 49
SLOT = 128              # edges per slot (one gather column)
GATHER_SLOTS = 64       # max slots per dma_gather instruction (8192 idx)

# x' row layout: rows 0..32766 = nodes 0..32766, row 32767 = zeros,
# rows 32768.. = nodes 32767..50175 (trailing node rows >= 50000 are zero).
LO_NODES = 32767        # nodes [0, 32767) live in the lo half
XROWS = 50304           # 32768 + 17536 (hi region padded with zeros)
HI_ROWS = XROWS - 32768
ZERO_LO = 32767         # lo-half zero row
ZERO_HI = 50000 - LO_NODES   # node 50000 (zero row) relative to hi base

BF16 = ml_dtypes.bfloat16

_last_results = None    # test.py introspects exec time from here


# ---------------------------------------------------------------------------
# Host-side preprocessing
# ---------------------------------------------------------------------------

def _prep_inputs(x, edge_index, w_embed, b_embed, w_assign, b_assign,
                 w_lin, b_lin):
    src = np.asarray(edge_index[0], dtype=np.int64)
    dst = np.asarray(edge_index[1], dtype=np.int64)

    deg = np.bincount(dst, minlength=N_NODES).astype(np.float64) + 1.0
    dinv_n = (1.0 / np.sqrt(deg)).astype(np.float32)        # [N]
    dinv = np.zeros(NPAD, dtype=np.float32)
    dinv[:N_NODES] = dinv_n

    loops = np.arange(N_NODES, dtype=np.int64)
    e_src = np.concatenate([src, loops])
    e_dst = np.concatenate([dst, loops])

    core_of = (e_dst // CORE_NODES).astype(np.int64)
    dloc = e_dst - core_of * CORE_NODES
    blk = dloc // P
    dstl = (dloc - blk * P).astype(np.int64)                # 0..127
    half = (e_src >= LO_NODES).astype(np.int64)             # 0=lo, 1=hi
    rel_src = (e_src - half * LO_NODES).astype(np.int64)    # int16-safe

    # rank of each edge within its (core, half, block, dstl) group
    order = np.lexsort((rel_src, dstl, blk, half, core_of))
    co, ho, bo, do, ro = (a[order] for a in
                          (core_of, half, blk, dstl, rel_src))
    gkey = ((co * 2 + ho) * NBLK + bo) * P + do
    firsts = np.r_[True, gkey[1:] != gkey[:-1]]
    gstart = np.flatnonzero(firsts)
    rank = (np.arange(gkey.size)
            - np.repeat(gstart, np.diff(np.r_[gstart, gkey.size])))

    # per (core, half, blk, dstl) degree
    degs = np.zeros((NCORES, 2, NBLK, P), dtype=np.int64)
    np.add.at(degs, (co, ho, bo, do), 1)

    # choose identity-rank R and tail slot count T per (half, blk):
    # identity slots cost dma+pe only; tail slots additionally cost the
    # DVE one-hot build.
    A, B = 142.0, 133.0
    R = np.zeros((2, NBLK), dtype=np.int64)
    T = np.zeros((2, NBLK), dtype=np.int64)
    for h in range(2):
        for b in range(NBLK):
            d_cb = degs[:, h, b, :]                         # [NCORES, P]
            maxd = int(d_cb.max())
            edges_c = d_cb.sum(axis=1)                      # [NCORES]
            best, r_pick, t_pick = None, 1, 0
            for r in range(maxd + 1):
                cov = np.minimum(d_cb, r).sum(axis=1)
                tail = edges_c - cov
                t = int(-(-int(tail.max()) // P)) if tail.max() > 0 else 0
                if r + t == 0:
                    continue
                cost = A * (r + t) + B * t
                if best is None or cost < best:
                    best, r_pick, t_pick = cost, r, t
            R[h, b], T[h, b] = r_pick, t_pick
    run_slots = R + T
    total_slots = int(run_slots.sum())
    half_slots = [int(run_slots[0].sum()), int(run_slots[1].sum())]
    run_off = np.zeros((2, NBLK), dtype=np.int64)
    acc = 0
    for h in range(2):
        for b in range(NBLK):
            run_off[h, b] = acc
            acc += run_slots[h, b]

    n_edge_slots = total_slots * SLOT
    idx_a = np.zeros((NCORES, n_edge_slots), dtype=np.int16)
    dstl_a = np.zeros((NCORES, n_edge_slots), dtype=np.float32)
    # pads point at the half's zero row so they contribute nothing
    for h in range(2):
        z = ZERO_LO if h == 0 else ZERO_HI
        for b in range(NBLK):
            o = run_off[h, b] * SLOT
            n = run_slots[h, b] * SLOT
            idx_a[:, o:o + n] = z

    # identity slots (rank < R) at partition dstl; tail edges packed into
    # the T tail slots in arrival order
    Rg = R[ho, bo]
    is_id = rank < Rg
    slot_pos_id = (run_off[ho, bo] + rank) * SLOT + do
    tkey = (co * 2 + ho) * NBLK + bo
    tmask = ~is_id
    tk = tkey[tmask]
    torder = np.argsort(tk, kind="stable")
    tk_sorted = tk[torder]
    tfirst = np.r_[True, tk_sorted[1:] != tk_sorted[:-1]]
    tstart = np.flatnonzero(tfirst)
    trank_sorted = (np.arange(tk_sorted.size)
                    - np.repeat(tstart, np.diff(np.r_[tstart,
                                                      tk_sorted.size])))
    trank = np.zeros(tk.size, dtype=np.int64)
    trank[torder] = trank_sorted
    tpos = np.zeros(len(gkey), dtype=np.int64)
    tpos[tmask] = (run_off[ho, bo][tmask] + Rg[tmask]) * SLOT + trank
    slot_pos = np.where(is_id, slot_pos_id, tpos)

    idx_a[co, slot_pos] = ro.astype(np.int16)
    dstl_a[co, slot_pos] = do.astype(np.float32)

    def to_lane_tiles(a, dt):
        t = a.reshape(NCORES, total_slots, SLOT).transpose(0, 2, 1)
        return np.ascontiguousarray(t.astype(dt))

    dstl_t = to_lane_tiles(dstl_a, BF16)                     # [8,128,S]

    idx16 = idx_a.reshape(NCORES, total_slots * SLOT // 16, 16)
    idx16 = idx16.transpose(0, 2, 1)                         # [8,16,S*8]
    idx_t = np.ascontiguousarray(np.tile(idx16, (1, 8, 1)))  # [8,128,S*8]

    # x' = dinv * x with the reserved lo zero row
    xs = np.asarray(x, dtype=np.float32) * dinv_n[:, None]
    xp = np.zeros((XROWS, IN_CH), dtype=BF16)
    xp[:LO_NODES] = xs[:LO_NODES].astype(BF16)
    xp[32768:32768 + (N_NODES - LO_NODES)] = xs[LO_NODES:].astype(BF16)

    wcat = np.concatenate([np.asarray(w_embed, np.float32),
                           np.asarray(w_assign, np.float32)],
                          axis=1).astype(BF16)               # [128,192]
    wlin_aug = np.concatenate([np.asarray(w_lin, np.float32),
                               np.asarray(b_lin, np.float32)[None, :]],
                              axis=0).astype(BF16)           # [65,512]
    bz_row = np.asarray(b_embed, np.float32).astype(BF16)[None, :]
    ones_row = np.ones((1, P), dtype=BF16)
    ba_col = np.asarray(b_assign, np.float32)[:, None].copy()
    iota = np.tile(np.arange(P, dtype=np.float32).astype(BF16)[None, :],
                   (P, 1)).copy()                            # [128,128]
    ident = np.eye(P, dtype=np.float32).astype(BF16)         # [128,128]

    masks = np.zeros((NCORES, P, NBLK), dtype=BF16)
    dinvmat = np.zeros((NCORES, P, CORE_NODES), dtype=BF16)
    for c in range(NCORES):
        nid = c * CORE_NODES + np.arange(CORE_NODES)
        dinvmat[c] = np.tile(dinv[nid][None, :].astype(BF16), (P, 1))
        nid2 = (c * CORE_NODES + np.arange(NBLK)[None, :] * P
                + np.arange(P)[:, None])
        masks[c] = (nid2 < N_NODES).astype(BF16)

    in_maps = []
    for c in range(NCORES):
        in_maps.append({
            "xp": xp,
            "idx": idx_t[c],
            "dstl": dstl_t[c],
            "wcat": wcat,
            "wlin": wlin_aug,
            "bz": bz_row,
            "ones": ones_row,
            "ba": ba_col,
            "iota": iota,
            "ident": ident,
            "dinvmat": dinvmat[c],
            "mask": masks[c],
            "tok": np.zeros((1, P), np.float32),
        })

    cfg = {
        "total_slots": total_slots,
        "half_slots": half_slots,
        "R": R, "T": T,
        "run_slots": run_slots,
        "run_off": run_off,
    }
    return in_maps, cfg


# ---------------------------------------------------------------------------
# Device program
# ---------------------------------------------------------------------------

def _gather_pieces(cfg):
    """(half, abs_slot_start, nslots) per dma_gather instruction."""
    pieces = []
    base = 0
    for h in range(2):
        n = cfg["half_slots"][h]
        s = 0
        while s < n:
            take = min(GATHER_SLOTS, n - s)
            pieces.append((h, base + s, take))
            s += take
        base += n
    return pieces


def build_program(cfg, reps=1):
    import concourse.bass as bass
    import concourse.mybir as mybir
    import concourse.bacc as bacc
    import concourse.tile as tile

    f32 = mybir.dt.float32
    bf16 = mybir.dt.bfloat16
    i16 = mybir.dt.int16

    S_total = cfg["total_slots"]
    R, T = cfg["R"], cfg["T"]
    run_slots, run_off = cfg["run_slots"], cfg["run_off"]
    pieces = _gather_pieces(cfg)

    piece_of = np.zeros(S_total, dtype=np.int64)
    col_of = np.zeros(S_total, dtype=np.int64)
    for pi, (h, s0, ns) in enumerate(pieces):
        piece_of[s0:s0 + ns] = pi
        col_of[s0:s0 + ns] = np.arange(ns)

    nc = bacc.Bacc("TRN2", target_bir_lowering=False, debug=False,
                   enable_asserts=False)

    d_xp = nc.dram_tensor("xp", [XROWS, IN_CH], bf16,
                          kind="ExternalInput").ap()
    d_idx = nc.dram_tensor("idx", [P, S_total * 8], i16,
                           kind="ExternalInput").ap()
    d_dstl = nc.dram_tensor("dstl", [P, S_total], bf16,
                            kind="ExternalInput").ap()
    d_wcat = nc.dram_tensor("wcat", [P, 192], bf16, kind="ExternalInput").ap()
    d_wlin = nc.dram_tensor("wlin", [65, 512], bf16, kind="ExternalInput").ap()
    d_bz = nc.dram_tensor("bz", [1, P], bf16, kind="ExternalInput").ap()
    d_ones = nc.dram_tensor("ones", [1, P], bf16, kind="ExternalInput").ap()
    d_ba = nc.dram_tensor("ba", [ASSIGN_CH, 1], f32, kind="ExternalInput").ap()
    d_iota = nc.dram_tensor("iota", [P, P], bf16, kind="ExternalInput").ap()
    d_ident = nc.dram_tensor("ident", [P, P], bf16, kind="ExternalInput").ap()
    d_dinv = nc.dram_tensor("dinvmat", [P, CORE_NODES], bf16,
                            kind="ExternalInput").ap()
    d_mask = nc.dram_tensor("mask", [P, NBLK], bf16, kind="ExternalInput").ap()
    d_out = nc.dram_tensor("xpoolT", [P, N_CLUSTERS], f32,
                           kind="ExternalOutput").ap()
    d_tok = nc.dram_tensor("tok", [1, P], f32, kind="ExternalInput").ap()
    d_tok_out = nc.dram_tensor("tok_out", [1, P], f32,
                               kind="ExternalOutput").ap()

    with tile.TileContext(nc) as tc, ExitStack() as ctx:
        cpool = ctx.enter_context(tc.tile_pool(name="const", bufs=1))

        t_idx = cpool.tile([P, S_total * 8], i16)
        t_dstl = cpool.tile([P, S_total], bf16)
        t_wcat = cpool.tile([P, 192], bf16)
        t_wlin = cpool.tile([65, 512], bf16)
        t_bz = cpool.tile([1, P], bf16)
        t_ones = cpool.tile([1, P], bf16)
        t_ba = cpool.tile([ASSIGN_CH, 1], f32)
        t_iota = cpool.tile([P, P], bf16)
        t_ident = cpool.tile([P, P], bf16)
        t_dinv = cpool.tile([P, CORE_NODES], bf16)
        t_mask = cpool.tile([P, NBLK], bf16)
        t_xaT = cpool.tile([P, NBLK * P], bf16)

        t_tok = cpool.tile([1, P], f32)
        nc.sync.dma_start(out=t_tok[:], in_=d_tok)
        nc.vector.tensor_scalar_add(t_tok[:], t_tok[:], 1.0)
        nc.sync.dma_start(out=d_tok_out, in_=t_tok[:])

        for t, d in ((t_idx, d_idx), (t_dstl, d_dstl), (t_wcat, d_wcat),
                     (t_wlin, d_wlin), (t_bz, d_bz), (t_ones, d_ones),
                     (t_ba, d_ba), (t_iota, d_iota), (t_ident, d_ident),
                     (t_dinv, d_dinv), (t_mask, d_mask)):
            nc.sync.dma_start(out=t[:], in_=d)

        gpool = ctx.enter_context(tc.tile_pool(name="gather", bufs=2))
        spool = ctx.enter_context(tc.tile_pool(name="onehot", bufs=2))
        app = ctx.enter_context(
            tc.tile_pool(name="xaT_psum", bufs=1, space="PSUM"))
        zpp = ctx.enter_context(
            tc.tile_pool(name="z_psum", bufs=2, space="PSUM"))
        spp = ctx.enter_context(
            tc.tile_pool(name="s0_psum", bufs=2, space="PSUM"))
        lpp = ctx.enter_context(
            tc.tile_pool(name="logit_psum", bufs=2, space="PSUM"))
        ppp = ctx.enter_context(
            tc.tile_pool(name="pool_psum", bufs=1, space="PSUM"))
        hpool = ctx.enter_context(tc.tile_pool(name="head", bufs=2))

        x_lo = d_xp[0:32768, :]
        x_hi = d_xp[32768:XROWS, :]

        for _rep in range(reps):
            # ---------------- phase 1: aggregate xaT ----------------
            gtiles = {}
            stiles = {}
            for pi, (h, s0, ns) in enumerate(pieces):
                gbuf = gpool.tile([P, GATHER_SLOTS, IN_CH], bf16, tag="g")
                sbuf = spool.tile([P, GATHER_SLOTS, P], bf16, tag="s")
                gtiles[pi] = gbuf
                stiles[pi] = sbuf
                nidx = ns * SLOT
                nc.gpsimd.dma_gather(
                    out_ap=gbuf[:, 0:ns, :],
                    in_ap=(x_lo if h == 0 else x_hi),
                    idxs_ap=t_idx[:, s0 * 8:(s0 + ns) * 8],
                    num_idxs=nidx,
                    num_idxs_reg=nidx,
                    elem_size=IN_CH,
                    single_packet=False,
                )
                # one-hot S for tail slots inside this piece
                for b in range(NBLK):
                    t0 = int(run_off[h, b] + R[h, b])
                    t1 = int(run_off[h, b] + run_slots[h, b])
                    lo = max(t0, s0)
                    hi = min(t1, s0 + ns)
                    if lo >= hi:
                        continue
                    c0 = lo - s0
                    c1 = hi - s0
                    nc.vector.tensor_tensor(
                        out=sbuf[:, c0:c1, :],
                        in0=t_iota[:][:, None, :].to_broadcast(
                            [P, c1 - c0, P]),
                        in1=t_dstl[:, lo:hi].to_broadcast([P, c1 - c0, P]),
                        op=mybir.AluOpType.is_equal,
                    )

            # per (half, block): R identity matmuls + T one-hot matmuls
            for h in range(2):
                for b in range(NBLK):
                    o = int(run_off[h, b])
                    r = int(R[h, b])
                    n = int(run_slots[h, b])
                    t_acc = app.tile([P, P], f32, tag="xaT")
                    for j in range(n):
                        s_abs = o + j
                        pi = int(piece_of[s_abs])
                        col = int(col_of[s_abs])
                        rhs = (t_ident[:] if j < r
                               else stiles[pi][:, col, :])
                        nc.tensor.matmul(
                            out=t_acc[:],
                            lhsT=gtiles[pi][:, col, :],
                            rhs=rhs,
                            start=(j == 0),
                            stop=(j == n - 1),
                        )
                    dst = t_xaT[:, b * P:(b + 1) * P]
                    if h == 0:
                        nc.scalar.copy(out=dst, in_=t_acc[:])
                    else:
                        nc.vector.tensor_add(out=dst, in0=t_acc[:], in1=dst)
                        nc.vector.tensor_tensor(
                            out=dst, in0=dst,
                            in1=t_dinv[:, b * P:(b + 1) * P],
                            op=mybir.AluOpType.mult)

            # ---------------- phase 2: per-block head ----------------
            t_pool = ppp.tile([P, N_CLUSTERS], f32, tag="xpool")
            for b in range(NBLK):
                xaT_b = t_xaT[:, b * P:(b + 1) * P]

                t_z = zpp.tile([P, P], f32, tag="z")
                nc.tensor.matmul(out=t_z[:], lhsT=xaT_b,
                                 rhs=t_wcat[:, 0:HID_CH],
                                 start=True, stop=False)
                nc.tensor.matmul(out=t_z[:], lhsT=t_ones[:], rhs=t_bz[:],
                                 start=False, stop=True)

                t_s0 = spp.tile([ASSIGN_CH, P], f32, tag="s0")
                nc.tensor.matmul(out=t_s0[:], lhsT=t_wcat[:, HID_CH:192],
                                 rhs=xaT_b, start=True, stop=True)

                t_sa = hpool.tile([65, P], bf16, tag="s0aug")
                nc.vector.tensor_scalar(
                    out=t_sa[0:ASSIGN_CH, :], in0=t_s0[:],
                    scalar1=t_ba[:], scalar2=0.0,
                    op0=mybir.AluOpType.add, op1=mybir.AluOpType.max)
                nc.vector.memset(t_sa[ASSIGN_CH:65, :], 1.0)

                t_lg = lpp.tile([P, N_CLUSTERS], f32, tag="logits")
                nc.tensor.matmul(out=t_lg[:], lhsT=t_sa[:], rhs=t_wlin[:],
                                 start=True, stop=True)

                t_e = hpool.tile([P, N_CLUSTERS], bf16, tag="e")
                t_den = hpool.tile([P, 1], f32, tag="den")
                nc.scalar.activation(out=t_e[:], in_=t_lg[:],
                                     func=mybir.ActivationFunctionType.Exp,
                                     accum_out=t_den[:])

                t_rd = hpool.tile([P, 1], f32, tag="rden")
                nc.vector.reciprocal(out=t_rd[:], in_=t_den[:])
                t_rdm = hpool.tile([P, 1], f32, tag="rdenm")
                nc.vector.tensor_tensor(out=t_rdm[:], in0=t_rd[:],
                                        in1=t_mask[:, b:b + 1],
                                        op=mybir.AluOpType.mult)
                t_zd = hpool.tile([P, P], bf16, tag="zdiv")
                nc.vector.tensor_scalar(
                    out=t_zd[:], in0=t_z[:],
                    scalar1=0.0, scalar2=t_rdm[:],
                    op0=mybir.AluOpType.max, op1=mybir.AluOpType.mult)

                nc.tensor.matmul(out=t_pool[:], lhsT=t_zd[:], rhs=t_e[:],
                                 start=(b == 0), stop=(b == NBLK - 1))

            t_out = cpool.tile([P, N_CLUSTERS], f32)
            nc.scalar.copy(out=t_out[:], in_=t_pool[:])
            nc.sync.dma_start(out=d_out, in_=t_out[:])

    nc.compile()
    return nc


# ---------------------------------------------------------------------------
# Entry point
# ---------------------------------------------------------------------------

def kernel(x, edge_index, w_embed, b_embed, w_assign, b_assign,
           w_lin, b_lin):
    global _last_results
    from concourse import bass_utils

    in_maps, cfg = _prep_inputs(x, edge_index, w_embed, b_embed,
                                w_assign, b_assign, w_lin, b_lin)
    nc = build_program(cfg)

    trace = bool(int(os.environ.get("KERNEL_TRACE", "0")))
    res = bass_utils.run_bass_kernel_spmd(
        nc, in_maps, core_ids=list(range(NCORES)), trace=trace)
    _last_results = res

    acc = np.zeros((P, N_CLUSTERS), dtype=np.float64)
    for om in res.results:
        acc += om["xpoolT"].astype(np.float64)
    return np.ascontiguousarray(acc.T.astype(np.float32))
